# revision 8
# baseline (speedup 1.0000x reference)
"""AAFM sparse-attention kernel for 8 TRN2 NeuronCores.

Math (per batch b):
    qp = q @ Wq.T + bq ; kp = k @ Wk.T + bk ; vp = v @ Wv.T + bv
    q_sig = sigmoid(qp)
    exp_a = exp(-alpha * log2(Sk) * distances)        # [Sq, Sk]
    exp_k = exp(kp)                                   # [Sk, D]
    out   = q_sig * (exp_a @ (exp_k * vp)) / (exp_a @ exp_k)

Sharding: data-parallel over batch B=8, one batch per core. All matmuls are
batch-local; no collectives. Host-side work is layout only (transposes so the
contraction dim lands on SBUF partitions) plus folding alpha into one scalar.

On-chip layout per core:
  - lhsT for projections: qT/kT/vT [D, S] chunked [128d, c, 128s]
  - B = [exp_k * vp | exp_k]  resident SBUF [128, 16, 1024] (k on partitions)
  - exp_a^T tiles [128k, 16c, 128q] from distances^T; stationary operand of the
    attention matmul out[q, d'] = sum_k exp_a^T[k,q].T @ B[k,d']
  - sigmoid via tanh (same ScalarE table set as exp -> no table thrash):
    sigmoid(x) = 0.5*tanh(x/2) + 0.5
Matmuls run as float32r (full PE rate at N=512, fp32 storage).
"""

import math
import sys

import numpy as np

sys.path.insert(0, "/opt/trn_rl_repo")

import concourse.bass as bass  # noqa: E402
import concourse.tile as tile  # noqa: E402
from concourse import bacc, mybir  # noqa: E402
from concourse.bass_utils import run_bass_kernel_spmd  # noqa: E402

P = 128
D = 512
S = 2048
B = 8
N_CORES = 8
DC = D // P  # 4 contraction chunks for projections

F32 = mybir.dt.float32
F32R = mybir.dt.float32r
BF16 = mybir.dt.bfloat16
AF = mybir.ActivationFunctionType
ALU = mybir.AluOpType


def build_graph(exp_scale: float, s: int = S, mm_dtype=BF16):
    """Build the single-core Bass/Tile graph. Same graph runs SPMD on 8 cores."""
    nt = s // P  # s-tiles == k-chunks == q-tiles
    nc = bacc.Bacc(
        "TRN2",
        target_bir_lowering=False,
        debug=False,
        enable_asserts=True,
        num_devices=N_CORES,
    )

    qT = nc.dram_tensor("qT", [D, s], F32, kind="ExternalInput").ap()
    kT = nc.dram_tensor("kT", [D, s], F32, kind="ExternalInput").ap()
    vT = nc.dram_tensor("vT", [D, s], F32, kind="ExternalInput").ap()
    dT = nc.dram_tensor("dT", [s, s], F32, kind="ExternalInput").ap()
    wq = nc.dram_tensor("wq", [D, D], F32, kind="ExternalInput").ap()
    wk = nc.dram_tensor("wk", [D, D], F32, kind="ExternalInput").ap()
    wv = nc.dram_tensor("wv", [D, D], F32, kind="ExternalInput").ap()
    bq = nc.dram_tensor("bq", [P, D], F32, kind="ExternalInput").ap()
    bk = nc.dram_tensor("bk", [P, D], F32, kind="ExternalInput").ap()
    bv = nc.dram_tensor("bv", [P, D], F32, kind="ExternalInput").ap()
    out = nc.dram_tensor("out", [s, D], F32, kind="ExternalOutput").ap()

    qT_r = qT.rearrange("(c p) s -> p c s", p=P)
    kT_r = kT.rearrange("(c p) s -> p c s", p=P)
    vT_r = vT.rearrange("(c p) s -> p c s", p=P)
    dT_r = dT.rearrange("(c p) q -> p c q", p=P)
    out_r = out.rearrange("(t p) e -> t p e", p=P)

    def mm(ps_ap, lhsT, rhs, start, stop):
        nc.tensor.matmul(ps_ap, lhsT, rhs, start=start, stop=stop)

    with tile.TileContext(nc) as tc:
        with (
            tc.tile_pool(name="consts", bufs=1) as consts,
            tc.tile_pool(name="resident", bufs=1) as resident,
            tc.tile_pool(name="stageA", bufs=3) as stageA,
            tc.tile_pool(name="stageB", bufs=2) as stageB,
            tc.tile_pool(name="tmpA", bufs=3) as tmpA,
            tc.tile_pool(name="tmpB", bufs=2) as tmpB,
            tc.tile_pool(name="outp", bufs=2) as outp,
            tc.tile_pool(name="psA", bufs=4, space="PSUM") as psA,
            tc.tile_pool(name="psB", bufs=2, space="PSUM") as psB,
        ):
            w_sb = {}
            for name, drm in (("wq", wq), ("wk", wk), ("wv", wv)):
                t = consts.tile([P, DC, D], mm_dtype, tag=f"w_{name}")
                # SWDGE DMA casts f32 -> bf16 inline
                nc.gpsimd.dma_start(t[:], drm.rearrange("(c p) e -> p c e", p=P))
                w_sb[name] = t
            b_sb = {}
            for name, drm in (("bq", bq), ("bk", bk), ("bv", bv)):
                t = consts.tile([P, D], F32, tag=f"b_{name}")
                nc.sync.dma_start(t[:], drm[:])
                b_sb[name] = t

            # B = [exp_k*vp | exp_k], k on partitions, chunk i holds rows
            # k = i*128+p. TQ = tanh(qp/2) per q-tile.
            Bm = resident.tile([P, nt, 2 * D], mm_dtype)
            TQ = resident.tile([P, nt, D], F32)

            # ---- Phase A: projections, exp_k, B build ----
            for i in range(nt):
                qa = stageA.tile([P, DC, P], mm_dtype, tag="qa")
                nc.gpsimd.dma_start(qa[:], qT_r[:, :, bass.ts(i, P)])
                ka = stageA.tile([P, DC, P], mm_dtype, tag="ka")
                nc.gpsimd.dma_start(ka[:], kT_r[:, :, bass.ts(i, P)])
                va = stageA.tile([P, DC, P], mm_dtype, tag="va")
                nc.gpsimd.dma_start(va[:], vT_r[:, :, bass.ts(i, P)])

                kp_ps = psA.tile([P, D], F32, tag="ps")
                vp_ps = psA.tile([P, D], F32, tag="ps")
                qp_ps = psA.tile([P, D], F32, tag="ps")
                for c in range(DC):
                    mm(kp_ps[:], ka[:, c, :], w_sb["wk"][:, c, :], c == 0, c == DC - 1)
                for c in range(DC):
                    mm(vp_ps[:], va[:, c, :], w_sb["wv"][:, c, :], c == 0, c == DC - 1)
                for c in range(DC):
                    mm(qp_ps[:], qa[:, c, :], w_sb["wq"][:, c, :], c == 0, c == DC - 1)

                # exp_k = exp(kp + bk) -> B[:, i, D:2D]
                kpb = tmpA.tile([P, D], F32, tag="kpb")
                nc.vector.tensor_add(kpb[:], kp_ps[:], b_sb["bk"][:])
                nc.scalar.activation(Bm[:, i, D : 2 * D], kpb[:], AF.Exp)
                # vp + bv, then exp_k * vp -> B[:, i, 0:D]
                vpb = tmpA.tile([P, D], F32, tag="vpb")
                nc.vector.tensor_add(vpb[:], vp_ps[:], b_sb["bv"][:])
                nc.vector.tensor_mul(Bm[:, i, 0:D], Bm[:, i, D : 2 * D], vpb[:])
                # tanh((qp + bq)/2) -> TQ[:, i, :]
                qpb = tmpA.tile([P, D], F32, tag="qpb")
                nc.vector.tensor_add(qpb[:], qp_ps[:], b_sb["bq"][:])
                nc.scalar.activation(TQ[:, i, :], qpb[:], AF.Tanh, scale=0.5)

            # ---- Phase B: exp_a, attention matmul, epilogue ----
            for j in range(nt):
                da = stageB.tile([P, nt, P], F32, tag="da")
                nc.sync.dma_start(da[:], dT_r[:, :, bass.ts(j, P)])
                ea = stageB.tile([P, nt, P], mm_dtype, tag="ea")
                nc.scalar.activation(ea[:], da[:], AF.Exp, scale=exp_scale)

                ps = psB.tile([P, 2, D], F32, tag="att")
                for h in range(2):
                    for c in range(nt):
                        mm(
                            ps[:, h, :],
                            ea[:, c, :],
                            Bm[:, c, bass.ts(h, D)],
                            c == 0,
                            c == nt - 1,
                        )

                r = tmpB.tile([P, D], F32, tag="recip")
                nc.vector.reciprocal_approx_fast(r[:], ps[:, 1, :])
                ath = tmpB.tile([P, D], F32, tag="ath")
                # ath = (num * 0.5) * (1/den)
                nc.vector.scalar_tensor_tensor(
                    ath[:], ps[:, 0, :], 0.5, r[:], op0=ALU.mult, op1=ALU.mult
                )
                ot = outp.tile([P, D], F32, tag="ot")
                # out = (tanh + 1) * ath  == sigmoid(qp) * num/den
                nc.vector.scalar_tensor_tensor(
                    ot[:], TQ[:, j, :], 1.0, ath[:], op0=ALU.add, op1=ALU.mult
                )
                nc.sync.dma_start(out_r[j], ot[:])

    nc.compile()
    return nc


def make_in_maps(q, k, v, distances, Wq, bq, Wk, bk, Wv, bv):
    """Per-core input maps: layout-only host work (transposes, bias tiling)."""
    wq_t = np.ascontiguousarray(Wq.T)  # [d, e]
    wk_t = np.ascontiguousarray(Wk.T)
    wv_t = np.ascontiguousarray(Wv.T)
    bq_t = np.ascontiguousarray(np.broadcast_to(bq[None, :], (P, D)))
    bk_t = np.ascontiguousarray(np.broadcast_to(bk[None, :], (P, D)))
    bv_t = np.ascontiguousarray(np.broadcast_to(bv[None, :], (P, D)))
    in_maps = []
    for b in range(B):
        in_maps.append(
            {
                "qT": np.ascontiguousarray(q[b].T),
                "kT": np.ascontiguousarray(k[b].T),
                "vT": np.ascontiguousarray(v[b].T),
                "dT": np.ascontiguousarray(distances[b].T),
                "wq": wq_t,
                "wk": wk_t,
                "wv": wv_t,
                "bq": bq_t,
                "bk": bk_t,
                "bv": bv_t,
            }
        )
    return in_maps


def _exp_scale(alpha, n):
    # mirror reference: log2_n = log(n)/log(2) in fp32, bias = -alpha*log2_n*d
    log2_n = np.float32(np.log(np.float32(n))) / np.float32(np.log(np.float32(2.0)))
    return float(np.float32(-np.float32(alpha) * log2_n))


_GRAPH_CACHE = {}


def run(q, k, v, distances, Wq, bq, Wk, bk, Wv, bv, alpha, trace=False, tmpdir=None):
    scale = _exp_scale(alpha[0], k.shape[1])
    key = scale
    if key not in _GRAPH_CACHE:
        _GRAPH_CACHE[key] = build_graph(scale)
    nc = _GRAPH_CACHE[key]
    in_maps = make_in_maps(q, k, v, distances, Wq, bq, Wk, bk, Wv, bv)
    res = run_bass_kernel_spmd(
        nc, in_maps, core_ids=list(range(N_CORES)), trace=trace, tmpdir=tmpdir
    )
    outs = np.stack([res.results[b]["out"] for b in range(B)], axis=0)
    return outs.astype(np.float32), res


def kernel(q, k, v, distances, Wq, bq, Wk, bk, Wv, bv, alpha):
    out, _ = run(q, k, v, distances, Wq, bq, Wk, bk, Wv, bv, alpha, trace=False)
    return out
